# revision 7
# baseline (speedup 1.0000x reference)
"""CASSI base-mode forward on 8 Trainium2 NeuronCores.

out[0, 0, m, l+n] = sum_l x[0, l, m, n] * ca[0, 0, m, n]
x: (1, 31, 1024, 1024) f32, ca: (1, 1, 1024, 1024) f32 -> out (1, 1, 1024, 1054) f32

Sharding: M (rows) split 8 ways -> 128 rows per core == SBUF partition count.
Each core: per band l, DMA x[l] shard, mask-multiply by ca shard (DVE),
shift-accumulate into a [128, 1054] accumulator, DMA out.
"""

from contextlib import ExitStack

import numpy as np

import concourse.bacc as bacc
import concourse.bass as bass
import concourse.mybir as mybir
import concourse.tile as tile
from concourse.bass_utils import run_bass_kernel_spmd

B, L, M, N = 1, 31, 1024, 1024
NCORES = 8
MS = M // NCORES  # 128 rows per core
W = N + L - 1  # 1054 output columns

_NC_CACHE = {}


def _build_nc():
    nc = bacc.Bacc()
    x_s = nc.dram_tensor("x_s", [L, MS, N], mybir.dt.float32, kind="ExternalInput")
    ca_s = nc.dram_tensor("ca_s", [MS, N], mybir.dt.float32, kind="ExternalInput")
    out_s = nc.dram_tensor("out_s", [MS, W], mybir.dt.float32, kind="ExternalOutput")

    f32 = mybir.dt.float32
    with tile.TileContext(nc) as tc:
        with (
            tc.tile_pool(name="cap", bufs=1) as cap,
            tc.tile_pool(name="xp", bufs=L) as xp,
            tc.tile_pool(name="accp", bufs=1) as accp,
        ):
            ca_t = cap.tile([MS, N], f32)
            nc.sync.dma_start(ca_t[:], ca_s[:])

            acc = accp.tile([MS, W], f32)
            nc.vector.memset(acc[:], 0.0)

            # The TensorTensor ISA struct has a single sync-wait slot; a tiny
            # read of ca_t here absorbs the ca-DMA wait so each band's mul
            # only needs to wait on its own x-band DMA queue.
            touch = cap.tile([MS, 1], f32)
            nc.vector.tensor_copy(touch[:], ca_t[:, 0:1])

            for l in range(L):
                xt = xp.tile([MS, N], f32)
                nc.sync.dma_start(xt[:], x_s[l])
                # mask multiply in place
                nc.vector.tensor_mul(xt[:], xt[:], ca_t[:])
                # shift-accumulate: out columns [l, l+N)
                nc.vector.tensor_add(acc[:, l : l + N], acc[:, l : l + N], xt[:])

            nc.gpsimd.dma_start(out_s[:], acc[:])
    nc.finalize()
    return nc


def make_in_maps(x, ca):
    in_maps = []
    for c in range(NCORES):
        rows = slice(c * MS, (c + 1) * MS)
        in_maps.append(
            {
                "x_s": np.ascontiguousarray(x[0, :, rows, :]),
                "ca_s": np.ascontiguousarray(ca[0, 0, rows, :]),
            }
        )
    return in_maps


def kernel(x, ca):
    x = np.ascontiguousarray(np.asarray(x, dtype=np.float32))
    ca = np.ascontiguousarray(np.asarray(ca, dtype=np.float32))
    assert x.shape == (B, L, M, N), x.shape
    assert ca.shape == (1, 1, M, N), ca.shape

    if "nc" not in _NC_CACHE:
        _NC_CACHE["nc"] = _build_nc()
    nc = _NC_CACHE["nc"]

    res = run_bass_kernel_spmd(nc, make_in_maps(x, ca), core_ids=list(range(NCORES)))
    out = np.concatenate([res.results[c]["out_s"] for c in range(NCORES)], axis=0)
    return out[None, None, :, :].astype(np.float32)


# revision 8
# speedup vs baseline: 1.0220x; 1.0220x over previous
"""CASSI base-mode forward on 8 Trainium2 NeuronCores (final).

Strategy (per core, M sharded 8 ways so each core owns 128 rows = the SBUF
partition count):
  - stream the 31 spectral bands of x from HBM (memory-bound: ~17 MiB/core)
  - DVE: mask-multiply each band by ca in place (exact fp32)
  - shift-accumulate band l into output columns [l, l+1024), split across
    engines to stay under the ~50 us HBM roofline:
      * PE: fp32 identity matmul accumulating into 2 pre-zeroed PSUM banks
        (exact -- weights are 1.0/0.0, fp32 2-pass decomposition is exact)
      * DVE: six whole bands + every band's last-30-column tail go to an
        SBUF accumulator
  - merge PSUM + SBUF accumulator on eviction, ACT triggers the output DMA.

out[0,0,m,l+n] = sum_l x[0,l,m,n] * ca[0,0,m,n]
Shard M across 8 cores (128 rows per core).
"""

import numpy as np

import concourse.bacc as bacc
import concourse.mybir as mybir
import concourse.tile as tile
from concourse.bass_utils import run_bass_kernel_spmd

B, L, M, N = 1, 31, 1024, 1024
NCORES = 8
MS = M // NCORES  # 128
W = N + L - 1  # 1054
BANK = 512

f32 = mybir.dt.float32
ALU = mybir.AluOpType

# bands whose full shift-add runs on DVE (rest: PE matmul + DVE 30-col tail)
DVE_BANDS = frozenset((5, 10, 15, 20, 25, 30))

_NC_CACHE = {}


def _build_nc():
    nc = bacc.Bacc()
    x_s = nc.dram_tensor("x_s", [L, MS, N], f32, kind="ExternalInput")
    ca_s = nc.dram_tensor("ca_s", [MS, N], f32, kind="ExternalInput")
    id_s = nc.dram_tensor("id_s", [MS, MS], f32, kind="ExternalInput")
    out_s = nc.dram_tensor("out_s", [MS, W], f32, kind="ExternalOutput")

    with tile.TileContext(nc) as tc:
        with (
            tc.tile_pool(name="cap", bufs=1) as cap,
            tc.tile_pool(name="xp", bufs=L) as xp,
            tc.tile_pool(name="outp", bufs=1) as outp,
            tc.tile_pool(name="ps", bufs=1, space="PSUM") as ps,
        ):
            ca_t = cap.tile([MS, N], f32)
            nc.sync.dma_start(ca_t[:], ca_s[:])
            id_t = cap.tile([MS, MS], f32)
            nc.sync.dma_start(id_t[:], id_s[:])

            zt = cap.tile([MS, BANK], f32)
            nc.vector.memset(zt[:], 0.0)

            # SBUF accumulator: full-band adds for DVE_BANDS + all 30-col tails
            acc = cap.tile([MS, W], f32)
            nc.vector.memset(acc[:], 0.0)

            pacc = ps.tile([MS, 2, BANK], f32)
            for k in range(2):
                nc.tensor.matmul(
                    pacc[:, k, :],
                    id_t[:],
                    zt[:],
                    start=True,
                    stop=False,
                    skip_group_check=True,
                )

            pe_bands = [l for l in range(L) if l not in DVE_BANDS]
            last_pe = pe_bands[-1]
            for l in range(L):
                xt = xp.tile([MS, N], f32)
                nc.sync.dma_start(xt[:], x_s[l])
                nc.vector.tensor_mul(xt[:], xt[:], ca_t[:])
                if l in DVE_BANDS:
                    # full shift-add on DVE (covers the tail too)
                    nc.vector.tensor_add(
                        acc[:, l : l + N], acc[:, l : l + N], xt[:]
                    )
                    continue
                # PE: cols [l, 2*BANK) in two bank-aligned segments
                a = l
                while a < min(l + N, 2 * BANK):
                    k = a // BANK
                    b = min((k + 1) * BANK, l + N, 2 * BANK)
                    nc.tensor.matmul(
                        pacc[:, k, a - k * BANK : b - k * BANK],
                        id_t[:],
                        xt[:, a - l : b - l],
                        start=False,
                        stop=(l == last_pe),
                        skip_group_check=True,
                    )
                    a = b
                # DVE: tail cols [2*BANK, l+N)
                if l > 0:
                    nc.vector.tensor_add(
                        acc[:, N : N + l], acc[:, N : N + l], xt[:, N - l : N]
                    )

            # merge: out = PSUM + acc on [0, 2*BANK), acc alone on the tail
            out_t = outp.tile([MS, W], f32)
            for k in range(2):
                nc.vector.scalar_tensor_tensor(
                    out_t[:, k * BANK : (k + 1) * BANK],
                    pacc[:, k, :],
                    1.0,
                    acc[:, k * BANK : (k + 1) * BANK],
                    ALU.mult,
                    ALU.add,
                )
            nc.scalar.copy(out_t[:, 2 * BANK : W], acc[:, 2 * BANK : W])
            nc.scalar.dma_start(out_s[:], out_t[:])
    nc.finalize()
    return nc


def make_in_maps(x, ca):
    ident = np.eye(MS, dtype=np.float32)
    in_maps = []
    for c in range(NCORES):
        rows = slice(c * MS, (c + 1) * MS)
        in_maps.append(
            {
                "x_s": np.ascontiguousarray(x[0, :, rows, :]),
                "ca_s": np.ascontiguousarray(ca[0, 0, rows, :]),
                "id_s": ident,
            }
        )
    return in_maps


def kernel(x, ca):
    x = np.ascontiguousarray(np.asarray(x, dtype=np.float32))
    ca = np.ascontiguousarray(np.asarray(ca, dtype=np.float32))
    assert x.shape == (B, L, M, N), x.shape
    assert ca.shape == (1, 1, M, N), ca.shape

    if "nc" not in _NC_CACHE:
        _NC_CACHE["nc"] = _build_nc()
    nc = _NC_CACHE["nc"]

    res = run_bass_kernel_spmd(nc, make_in_maps(x, ca), core_ids=list(range(NCORES)))
    out = np.concatenate([res.results[c]["out_s"] for c in range(NCORES)], axis=0)
    return out[None, None, :, :].astype(np.float32)
